# revision 4
# baseline (speedup 1.0000x reference)
"""Trainium2 Bass kernel for the MatchMatrix pairwise MLP.

kernel(**inputs) takes the FULL inputs (sent_a/sent_b [8,128,256], W1 [512,64],
b1, W2, b2, W3, b3) and returns the FULL [8,128,128,16] float32 output.

Sharding: data-parallel over batch B=8 -> one batch element per NeuronCore
(8 cores); weights/biases are replicated. Each core runs a fused Bass/Tile
kernel and the per-core [128,128,16] outputs are stacked.

Per-core design (v2 — built around the PE p-state ramp):
  - PE hits 2.4GHz only after 3us of gapless execution (1.2GHz otherwise),
    so the kernel opens with n_warm dependency-free dummy matmuls (memset
    operands, no ident needed) that overlap the input-DMA window, then chains
    transposes -> bias cols -> a2 -> b_pairs -> 4 psO b3-seeds -> L2/L3 loop
    with no PE idle gaps.
  - L1 (h1 = leaky(a2 (+) b)) is fp16 end to end with free order (a, jj) so
    every DVE operand has a packed innermost dim (enables DVE 2x/4x modes):
    one broadcast add (TENSOR_TENSOR) + one leaky (SCALAR_TENSOR_TENSOR) per
    chunk of 1-2 t-groups, instead of 16x small fp32 broadcast ops.
  - L2: ps2[128=(u,e,k1), 512=(a,jj)] = blockdiag(W2,W2).T @ h1a (2 mm/t).
  - h2 evict on ACT (bias b2q + PReLU + fp32->fp16 in one pass).
  - L3 fused with output transpose: psO[128=a, 64=(c,k2)] per (t,jj) with
    strided lhsT; psO banks pre-seeded with b3 rows (K=1 matmuls) in the head.
  - Final leaky: s0/s1 on ACT, s2/s3 on DVE; output DMAs on the Sync queue.
Matmul operands are fp16 (fp32 accumulation in PSUM).
"""
import sys
import numpy as np

for _p in ("/opt/trn_rl_repo", "/root/.axon_site/_ro/trn_rl_repo"):
    if _p not in sys.path:
        sys.path.append(_p)

from contextlib import ExitStack

import concourse.bass as bass
import concourse.tile as tile
from concourse import bacc, mybir, masks
from concourse import bass_utils

F32 = mybir.dt.float32
FP16 = mybir.dt.float16
AF = mybir.ActivationFunctionType
ALU = mybir.AluOpType

# t -> (chunk index, col base within chunk); chunks hold 1,1,2,2,2 t's
CHUNK_OF_T = [(0, 0), (1, 0), (2, 0), (2, 1024), (3, 0), (3, 1024), (4, 0), (4, 1024)]
CHUNK_G0_NG = [(0, 2), (2, 2), (4, 4), (8, 4), (12, 4)]  # (first group, n groups)


def build_nc(act: str = "prelu", n_warm: int = 5):
    nc = bacc.Bacc("TRN2", target_bir_lowering=False, debug=False, num_devices=8)
    sa = nc.dram_tensor("sent_a", [128, 256], F32, kind="ExternalInput").ap()
    sb = nc.dram_tensor("sent_b", [128, 256], F32, kind="ExternalInput").ap()
    W1 = nc.dram_tensor("W1", [512, 64], F32, kind="ExternalInput").ap()
    b1 = nc.dram_tensor("b1", [64], F32, kind="ExternalInput").ap()
    W2 = nc.dram_tensor("W2", [64, 32], F32, kind="ExternalInput").ap()
    b2 = nc.dram_tensor("b2", [32], F32, kind="ExternalInput").ap()
    W3 = nc.dram_tensor("W3", [32, 16], F32, kind="ExternalInput").ap()
    b3 = nc.dram_tensor("b3", [16], F32, kind="ExternalInput").ap()
    out = nc.dram_tensor("out", [128, 128, 16], F32, kind="ExternalOutput").ap()

    with tile.TileContext(nc) as tc, ExitStack() as ctx:
        _body(ctx, tc, sa, sb, W1, b1, W2, b2, W3, b3, out, act, n_warm)
    nc.compile()
    return nc


def _body(ctx, tc, sa, sb, W1, b1, W2, b2, W3, b3, out, act, n_warm):
    nc = tc.nc
    mm_dt = FP16
    alpha = 0.01 if act == "prelu" else 0.0

    def act_leaky(out_ap, in_ap, bias):
        if act == "prelu":
            nc.scalar.activation(out_ap, in_ap, AF.Prelu, bias=bias, alpha=alpha)
        else:
            nc.scalar.activation(out_ap, in_ap, AF.Relu, bias=bias)

    def dve_leaky(out_ap, in_ap):
        nc.vector.scalar_tensor_tensor(
            out=out_ap, in0=in_ap, scalar=alpha, in1=in_ap,
            op0=ALU.mult, op1=ALU.max)

    const = ctx.enter_context(tc.tile_pool(name="const", bufs=1))
    work = ctx.enter_context(tc.tile_pool(name="work", bufs=6))
    h1pool = ctx.enter_context(tc.tile_pool(name="h1p", bufs=3))
    psS = ctx.enter_context(tc.tile_pool(name="psS", bufs=2, space="PSUM"))
    psA = ctx.enter_context(tc.tile_pool(name="psA", bufs=2, space="PSUM"))
    psC = ctx.enter_context(tc.tile_pool(name="psC", bufs=4, space="PSUM"))

    # ---------- GpSimd (Pool) queue head: memsets, ident, W2/W3 DMAs ----------
    dmy_l = const.tile([128, 128], mm_dt, tag="dmyl")
    nc.gpsimd.memset(dmy_l[:], 0.0)
    dmy_r = const.tile([128, 512], mm_dt, tag="dmyr")
    nc.gpsimd.memset(dmy_r[:], 0.0)
    ones1 = const.tile([1, 1], F32, tag="ones1")
    nc.gpsimd.memset(ones1[:], 1.0)
    ones_col = const.tile([1, 128], mm_dt, tag="onescol")
    nc.gpsimd.memset(ones_col[:], 1.0)
    ident = const.tile([128, 128], F32, tag="ident")
    masks.make_identity(nc, ident[:])
    W2st = const.tile([64, 32], F32, tag="w2st")
    nc.gpsimd.dma_start(W2st[:], W2[:])
    W3st = const.tile([32, 16], F32, tag="w3st")
    nc.gpsimd.dma_start(W3st[:], W3[:])
    W2dd = const.tile([128, 64], mm_dt, tag="w2dd")
    nc.gpsimd.memset(W2dd[:], 0.0)
    W3dd = const.tile([128, 64], mm_dt, tag="w3dd")
    nc.gpsimd.memset(W3dd[:], 0.0)
    # calibration freebie: Pool fp16 packed TT rate (runs in Pool dead time)
    calscr = const.tile([128, 512], mm_dt, tag="calscr")
    nc.gpsimd.tensor_tensor(out=calscr[:], in0=dmy_r[:], in1=dmy_r[:], op=ALU.add)

    # ---------- Scalar (ACT) queue head: bias DMAs, then table warm ----------
    b1r = const.tile([1, 64], F32, tag="b1r")
    nc.scalar.dma_start(b1r[:], b1[:].unsqueeze(0))
    b2r = const.tile([1, 32], F32, tag="b2r")
    nc.scalar.dma_start(b2r[:], b2[:].unsqueeze(0))
    b3r = const.tile([1, 16], F32, tag="b3r")
    nc.scalar.dma_start(b3r[:], b3[:].unsqueeze(0))
    warm2 = const.tile([1, 1], F32, tag="warm2")
    act_leaky(warm2[:], ones1[0:1, 0:1], 0.0)

    # ---------- Sync queue: input DMAs ----------
    sa_sb = const.tile([128, 256], F32, tag="sa")
    nc.sync.dma_start(sa_sb[:], sa[:])
    sb_sb = const.tile([128, 256], F32, tag="sb")
    nc.sync.dma_start(sb_sb[:], sb[:])
    W1v = W1.rearrange("(c p) k -> p c k", p=128)
    W1all = const.tile([128, 256], F32, tag="w1all")
    nc.sync.dma_start(
        W1all[:, 0:128].rearrange("p (c k) -> p c k", c=2), W1v[:, 0:2, :])
    nc.sync.dma_start(
        W1all[:, 128:256].rearrange("p (c k) -> p c k", c=2), W1v[:, 2:4, :])

    # ---------- PE ramp: gapless dummy matmuls over the DMA window ----------
    for _w in range(n_warm):
        dps = psS.tile([128, 512], F32, tag="pst")
        nc.tensor.matmul(dps[:], dmy_l[:], dmy_r[:], start=True, stop=True)

    # ---------- DVE head: bias rows, weight casts (after DMAs land) ----------
    b1rep = const.tile([1, 128], F32, tag="b1rep")
    nc.vector.tensor_copy(
        b1rep[:].rearrange("o (r k) -> o r k", r=2),
        b1r[:].unsqueeze(1).broadcast_to([1, 2, 64]))
    b2rep = const.tile([1, 128], F32, tag="b2rep")
    nc.vector.tensor_copy(
        b2rep[:].rearrange("o (r k) -> o r k", r=4),
        b2r[:].unsqueeze(1).broadcast_to([1, 4, 32]))
    b3row64 = const.tile([1, 64], F32, tag="b3row64")
    nc.vector.tensor_copy(
        b3row64[:].rearrange("o (c k) -> o c k", c=4),
        b3r[:].unsqueeze(1).broadcast_to([1, 4, 16]))
    b3rep = const.tile([1, 512], mm_dt, tag="b3rep")
    nc.vector.tensor_copy(
        b3rep[:].rearrange("o (r w) -> o r w", r=8),
        b3row64[:].unsqueeze(1).broadcast_to([1, 8, 64]))
    Wa_dup = []
    for c in (0, 1):
        w = const.tile([128, 128], mm_dt, tag=f"wadup{c}")
        srcv = W1all[:, 64 * c : 64 * c + 64].unsqueeze(1).broadcast_to([128, 2, 64])
        nc.vector.tensor_copy(w[:].rearrange("p (d k) -> p d k", d=2), srcv)
        Wa_dup.append(w)
    Wbh = const.tile([128, 128], mm_dt, tag="wbh")
    nc.vector.tensor_copy(Wbh[:], W1all[:, 128:256])
    nc.vector.tensor_copy(W2dd[0:64, 0:32], W2st[:])
    nc.vector.tensor_copy(W2dd[64:128, 32:64], W2st[:])
    for c in range(4):
        nc.vector.tensor_copy(W3dd[32 * c : 32 * c + 32, 16 * c : 16 * c + 16], W3st[:])

    # ---------- PE: transposes (+DVE casts), bias cols, a2, b_pairs ----------
    saT, sbT = [], []
    for src, dstlist, nm in ((sa_sb, saT, "saT"), (sb_sb, sbT, "sbT")):
        for c in (0, 1):
            ps = psS.tile([128, 128], F32, tag="pst")
            nc.tensor.transpose(ps[:], src[:, 128 * c : 128 * (c + 1)], ident[:])
            t = work.tile([128, 128], mm_dt, tag=f"{nm}{c}")
            nc.vector.tensor_copy(t[:], ps[:])
            dstlist.append(t)

    def bias_col(row_ap, width, tag):
        # row_ap [1, w] -> column [w, 1] via a K=1 outer product
        ps = psS.tile([128, 128], F32, tag="pst")
        nc.tensor.matmul(ps[0:width, 0:1], row_ap, ones1[:], start=True, stop=True)
        col = const.tile([width, 1], F32, tag=tag)
        nc.vector.tensor_copy(col[:], ps[0:width, 0:1])
        return col

    b1d = bias_col(b1rep[:], 128, "b1d")    # [128,1]: (e,k0)
    b2q = bias_col(b2rep[:], 128, "b2q")    # [128,1]: (u,e,k1)

    ps_a2 = psS.tile([128, 128], F32, tag="pst")
    nc.tensor.matmul(ps_a2[:], Wa_dup[0][:], saT[0][:], start=True, stop=False)
    nc.tensor.matmul(ps_a2[:], Wa_dup[1][:], saT[1][:], start=False, stop=True)
    a2sb = work.tile([128, 128], mm_dt, tag="a2sb")
    nc.scalar.activation(a2sb[:], ps_a2[:], AF.Identity, bias=b1d[:, 0:1])
    # a2rep [128, 512=(a,jj)] = a2 repeated 4x along the inner jj dim
    a2rep = const.tile([128, 512], mm_dt, tag="a2rep")
    nc.vector.tensor_copy(
        a2rep[:].rearrange("p (a j) -> p a j", j=4),
        a2sb[:].unsqueeze(2).broadcast_to([128, 128, 4]))

    ps_bT = psS.tile([64, 128], F32, tag="pst")
    nc.tensor.matmul(ps_bT[:], Wbh[:, 0:64], sbT[0][:], start=True, stop=False)
    nc.tensor.matmul(ps_bT[:], Wbh[:, 64:128], sbT[1][:], start=False, stop=True)
    # b_pairs2 [128=(e,k0), 64=(t,u,jj)]: col q' = 8t+4u+jj, j = 16t+4jj+2u+e
    b_pairs2 = const.tile([128, 64], mm_dt, tag="bpairs2")
    bTv = ps_bT[:].rearrange("p (t j u e) -> p t u j e", t=8, j=4, u=2)
    for e in (0, 1):
        nc.vector.tensor_copy(
            b_pairs2[64 * e : 64 * e + 64, :].rearrange(
                "p (t u j) -> p t u j", t=8, u=2),
            bTv[:, :, :, :, e])

    # ---------- PE: pre-seed all 4 psO banks with the b3 rows ----------
    psO_list = []
    for s in range(4):
        psO = psC.tile([128, 512], F32, tag="psO")
        nc.tensor.matmul(psO[:], ones_col[:], b3rep[:],
                         start=True, stop=False, skip_group_check=True)
        psO_list.append(psO)

    # ---------- L1 chunks: fp16 broadcast add + leaky, packed innermost ----
    def emit_chunk(ci):
        g0, ng = CHUNK_G0_NG[ci]
        w = 512 * ng
        h1 = h1pool.tile([128, w], mm_dt, tag=f"h1_{ng}")
        in0 = (a2rep[:].rearrange("p (a j) -> p a j", j=4)
               .unsqueeze(1).broadcast_to([128, ng, 128, 4]))
        in1 = (b_pairs2[:, 4 * g0 : 4 * (g0 + ng)]
               .rearrange("p (g j) -> p g j", g=ng)
               .unsqueeze(2).broadcast_to([128, ng, 128, 4]))
        nc.vector.tensor_tensor(
            out=h1[:].rearrange("p (g a j) -> p g a j", g=ng, j=4),
            in0=in0, in1=in1, op=ALU.add)
        h1a = h1pool.tile([128, w], mm_dt, tag=f"h1a_{ng}")
        dve_leaky(h1a[:], h1[:])
        return h1a

    chunks = {0: emit_chunk(0), 1: emit_chunk(1), 2: emit_chunk(2)}

    out_flat = out.rearrange("a j k -> a (j k)")

    def l3_stage(t_, h2):
        psO = psO_list[t_ >> 1]
        h2v = h2[:].rearrange("p (a j) -> p j a", j=4)
        for jj in range(4):
            col = 256 * (t_ & 1) + 64 * jj
            last = (t_ % 2 == 1) and (jj == 3)
            nc.tensor.matmul(
                psO[:, col : col + 64], h2v[:, jj, :], W3dd[:],
                start=False, stop=last, skip_group_check=True)
        if t_ % 2 == 1:
            s = t_ >> 1
            osb = work.tile([128, 512], F32, tag="osb")
            if s < 3:
                act_leaky(osb[:], psO[:], 0.0)
            else:
                # STT can't read both inputs from PSUM: copy out, leaky in place
                nc.vector.tensor_copy(osb[:], psO[:])
                dve_leaky(osb[:], osb[:])
            nc.sync.dma_start(out_flat[:, 512 * s : 512 * (s + 1)], osb[:])

    # ---------- main L2 / L3 loop ----------
    h2_tiles = {}
    for t_ in range(8):
        ci, base = CHUNK_OF_T[t_]
        ps2 = psA.tile([128, 512], F32, tag="ps2")
        for u in (0, 1):
            nc.tensor.matmul(
                ps2[64 * u : 64 * u + 64, :], W2dd[:],
                chunks[ci][:, base + 512 * u : base + 512 * u + 512],
                start=True, stop=True)
        if t_ == 1:
            chunks[3] = emit_chunk(3)
        if t_ == 3:
            chunks[4] = emit_chunk(4)
        h2 = work.tile([128, 512], mm_dt, tag="h2")
        act_leaky(h2[:], ps2[:], b2q[:, 0:1])
        h2_tiles[t_] = h2
        if t_ >= 2:
            l3_stage(t_ - 2, h2_tiles.pop(t_ - 2))
    l3_stage(6, h2_tiles.pop(6))
    l3_stage(7, h2_tiles.pop(7))


_NC_CACHE = {}


def _get_nc():
    if "nc" not in _NC_CACHE:
        _NC_CACHE["nc"] = build_nc()
    return _NC_CACHE["nc"]


def kernel(sent_a, sent_b, W1, b1, W2, b2, W3, b3):
    sent_a = np.ascontiguousarray(np.asarray(sent_a, dtype=np.float32))
    sent_b = np.ascontiguousarray(np.asarray(sent_b, dtype=np.float32))
    W1 = np.ascontiguousarray(np.asarray(W1, dtype=np.float32))
    b1 = np.ascontiguousarray(np.asarray(b1, dtype=np.float32))
    W2 = np.ascontiguousarray(np.asarray(W2, dtype=np.float32))
    b2 = np.ascontiguousarray(np.asarray(b2, dtype=np.float32))
    W3 = np.ascontiguousarray(np.asarray(W3, dtype=np.float32))
    b3 = np.ascontiguousarray(np.asarray(b3, dtype=np.float32))

    nc = _get_nc()
    in_maps = [{
        "sent_a": sent_a[i], "sent_b": sent_b[i],
        "W1": W1, "b1": b1, "W2": W2, "b2": b2, "W3": W3, "b3": b3,
    } for i in range(8)]
    res = bass_utils.run_bass_kernel_spmd(nc, in_maps, core_ids=list(range(8)))
    return np.stack([res.results[i]["out"] for i in range(8)]).astype(np.float32)


# revision 6
# speedup vs baseline: 1.1821x; 1.1821x over previous
"""Trainium2 Bass kernel for the MatchMatrix pairwise MLP.

kernel(**inputs) takes the FULL inputs (sent_a/sent_b [8,128,256], W1 [512,64],
b1, W2, b2, W3, b3) and returns the FULL [8,128,128,16] float32 output.

Sharding: data-parallel over batch B=8 -> one batch element per NeuronCore
(8 cores); weights/biases are replicated.

Per-core design (v3):
  - leaky-relu split: leaky(x) = x + relu(-0.99x), so L1 materializes the raw
    pairwise pre-activation h1 = a2 (+) b (one DVE TENSOR_TENSOR, 2x mode) and
    n' = relu(-0.99 h1) (one DVE TENSOR_SCALAR, 4x mode); the leaky itself is
    absorbed into L2 as a second accumulating matmul with the same W2 weights.
    This removes the 1x-mode SCALAR_TENSOR_TENSOR leaky that dominated DVE.
  - PE p-state ramp: 5 dependency-free warm-up matmuls (memset operands) open
    the kernel gaplessly over the input-DMA window so the PE reaches 2.4GHz
    (3us continuous-execution requirement), then transposes / bias columns /
    a2 / b_pairs / psO b3-seeds chain into the L2/L3 loop without PE idle.
  - L1 free order (a, jj) keeps every DVE operand innermost-packed (2x/4x).
  - L2: ps2[128=(u,e,k1), 512=(a,jj)] = blockdiag(W2,W2).T @ (h1 then n').
  - h2 evict on ACT (bias b2q + PReLU + fp32->fp16 in one pass).
  - L3 fused with output transpose: psO[128=a, 64=(c,k2)] per (t,jj); psO
    banks pre-seeded with b3 rows (K=1 matmuls).
  - Final leaky: s0-s2 on ACT; s3 split ACT/DVE to shorten the tail; output
    DMAs on Sync (+1 on GpSimd for the last half-chunk).
  - Weight-prep copies and bias-column evictions run on the Pool queue to keep
    DVE free for the transpose casts and L1 chunks.
"""
import sys
import numpy as np

for _p in ("/opt/trn_rl_repo", "/root/.axon_site/_ro/trn_rl_repo"):
    if _p not in sys.path:
        sys.path.append(_p)

from contextlib import ExitStack

import concourse.bass as bass
import concourse.tile as tile
from concourse import bacc, mybir, masks
from concourse import bass_utils

F32 = mybir.dt.float32
FP16 = mybir.dt.float16
AF = mybir.ActivationFunctionType
ALU = mybir.AluOpType

# t -> (chunk index, col base within chunk); 4 chunks of 2 t's (2048 cols)
CHUNK_OF_T = [(0, 0), (0, 1024), (1, 0), (1, 1024), (2, 0), (2, 1024), (3, 0), (3, 1024)]


def build_nc(act: str = "prelu", n_warm: int = 5):
    nc = bacc.Bacc("TRN2", target_bir_lowering=False, debug=False, num_devices=8)
    sa = nc.dram_tensor("sent_a", [128, 256], F32, kind="ExternalInput").ap()
    sb = nc.dram_tensor("sent_b", [128, 256], F32, kind="ExternalInput").ap()
    W1 = nc.dram_tensor("W1", [512, 64], F32, kind="ExternalInput").ap()
    b1 = nc.dram_tensor("b1", [64], F32, kind="ExternalInput").ap()
    W2 = nc.dram_tensor("W2", [64, 32], F32, kind="ExternalInput").ap()
    b2 = nc.dram_tensor("b2", [32], F32, kind="ExternalInput").ap()
    W3 = nc.dram_tensor("W3", [32, 16], F32, kind="ExternalInput").ap()
    b3 = nc.dram_tensor("b3", [16], F32, kind="ExternalInput").ap()
    out = nc.dram_tensor("out", [128, 128, 16], F32, kind="ExternalOutput").ap()

    with tile.TileContext(nc) as tc, ExitStack() as ctx:
        _body(ctx, tc, sa, sb, W1, b1, W2, b2, W3, b3, out, act, n_warm)
    nc.compile()
    return nc


def _body(ctx, tc, sa, sb, W1, b1, W2, b2, W3, b3, out, act, n_warm):
    nc = tc.nc
    mm_dt = FP16
    alpha = 0.01 if act == "prelu" else 0.0
    # leaky(x) = x + relu(-(1-alpha)x)
    neg_slope = -(1.0 - alpha)

    def act_leaky(out_ap, in_ap, bias):
        if act == "prelu":
            nc.scalar.activation(out_ap, in_ap, AF.Prelu, bias=bias, alpha=alpha)
        else:
            nc.scalar.activation(out_ap, in_ap, AF.Relu, bias=bias)

    const = ctx.enter_context(tc.tile_pool(name="const", bufs=1))
    work = ctx.enter_context(tc.tile_pool(name="work", bufs=6))
    h1pool = ctx.enter_context(tc.tile_pool(name="h1p", bufs=3))
    psS = ctx.enter_context(tc.tile_pool(name="psS", bufs=3, space="PSUM"))
    psA = ctx.enter_context(tc.tile_pool(name="psA", bufs=2, space="PSUM"))
    psC = ctx.enter_context(tc.tile_pool(name="psC", bufs=3, space="PSUM"))

    # ---------- GpSimd (Pool) queue: memsets, ident, W2/W3 DMAs, W-prep ----
    dmy_l = const.tile([128, 128], mm_dt, tag="dmyl")
    nc.gpsimd.memset(dmy_l[:], 0.0)
    dmy_r = const.tile([128, 512], mm_dt, tag="dmyr")
    nc.gpsimd.memset(dmy_r[:], 0.0)
    ones1 = const.tile([1, 1], F32, tag="ones1")
    nc.gpsimd.memset(ones1[:], 1.0)
    ones_col = const.tile([1, 128], mm_dt, tag="onescol")
    nc.gpsimd.memset(ones_col[:], 1.0)
    ident = const.tile([128, 128], F32, tag="ident")
    masks.make_identity(nc, ident[:])
    W2st = const.tile([64, 32], F32, tag="w2st")
    nc.gpsimd.dma_start(W2st[:], W2[:])
    W3st = const.tile([32, 16], F32, tag="w3st")
    nc.gpsimd.dma_start(W3st[:], W3[:])
    W2dd = const.tile([128, 64], mm_dt, tag="w2dd")
    nc.gpsimd.memset(W2dd[:], 0.0)
    W3dd = const.tile([128, 64], mm_dt, tag="w3dd")
    nc.gpsimd.memset(W3dd[:], 0.0)

    # ---------- Scalar (ACT): table warm first, then bias DMAs ----------
    warm2 = const.tile([1, 1], F32, tag="warm2")
    act_leaky(warm2[:], ones1[0:1, 0:1], 0.0)
    b1r = const.tile([1, 64], F32, tag="b1r")
    nc.scalar.dma_start(b1r[:], b1[:].unsqueeze(0))
    b2r = const.tile([1, 32], F32, tag="b2r")
    nc.scalar.dma_start(b2r[:], b2[:].unsqueeze(0))
    b3r = const.tile([1, 16], F32, tag="b3r")
    nc.scalar.dma_start(b3r[:], b3[:].unsqueeze(0))

    # ---------- Sync queue: input DMAs (sa first: it gates the PE head) ----
    sa_sb = const.tile([128, 256], F32, tag="sa")
    nc.sync.dma_start(sa_sb[:], sa[:])
    sb_sb = const.tile([128, 256], F32, tag="sb")
    nc.sync.dma_start(sb_sb[:], sb[:])
    W1v = W1.rearrange("(c p) k -> p c k", p=128)
    W1all = const.tile([128, 256], F32, tag="w1all")
    nc.sync.dma_start(
        W1all[:, 0:128].rearrange("p (c k) -> p c k", c=2), W1v[:, 0:2, :])
    nc.sync.dma_start(
        W1all[:, 128:256].rearrange("p (c k) -> p c k", c=2), W1v[:, 2:4, :])

    # ---------- PE ramp: gapless dummy matmuls over the DMA window ----------
    for _w in range(n_warm):
        dps = psS.tile([128, 512], F32, tag="pst")
        nc.tensor.matmul(dps[:], dmy_l[:], dmy_r[:], start=True, stop=True)

    # ---------- DVE: bias rows (early arrivals) ----------
    b1rep = const.tile([1, 128], F32, tag="b1rep")
    nc.vector.tensor_copy(
        b1rep[:].rearrange("o (r k) -> o r k", r=2),
        b1r[:].unsqueeze(1).broadcast_to([1, 2, 64]))
    b2rep = const.tile([1, 128], F32, tag="b2rep")
    nc.vector.tensor_copy(
        b2rep[:].rearrange("o (r k) -> o r k", r=4),
        b2r[:].unsqueeze(1).broadcast_to([1, 4, 32]))
    b3row64 = const.tile([1, 64], F32, tag="b3row64")
    nc.vector.tensor_copy(
        b3row64[:].rearrange("o (c k) -> o c k", c=4),
        b3r[:].unsqueeze(1).broadcast_to([1, 4, 16]))
    b3rep = const.tile([1, 512], mm_dt, tag="b3rep")
    nc.vector.tensor_copy(
        b3rep[:].rearrange("o (r w) -> o r w", r=8),
        b3row64[:].unsqueeze(1).broadcast_to([1, 8, 64]))

    # ---------- Pool: weight casts (arrive mid-warmup) ----------
    Wa_dup = []
    for c in (0, 1):
        w = const.tile([128, 128], mm_dt, tag=f"wadup{c}")
        srcv = W1all[:, 64 * c : 64 * c + 64].unsqueeze(1).broadcast_to([128, 2, 64])
        nc.gpsimd.tensor_copy(w[:].rearrange("p (d k) -> p d k", d=2), srcv)
        Wa_dup.append(w)
    Wbh = const.tile([128, 128], mm_dt, tag="wbh")
    nc.gpsimd.tensor_copy(Wbh[:], W1all[:, 128:256])
    nc.gpsimd.tensor_copy(W2dd[0:64, 0:32], W2st[:])
    nc.gpsimd.tensor_copy(W2dd[64:128, 32:64], W2st[:])
    for c in range(4):
        nc.gpsimd.tensor_copy(W3dd[32 * c : 32 * c + 32, 16 * c : 16 * c + 16], W3st[:])

    # ---------- PE: transposes (+DVE casts), bias cols, a2, b_pairs --------
    saT, sbT = [], []

    def transpose_pair(src, dstlist, nm):
        for c in (0, 1):
            ps = psS.tile([128, 128], F32, tag="pst")
            nc.tensor.transpose(ps[:], src[:, 128 * c : 128 * (c + 1)], ident[:])
            t = work.tile([128, 128], mm_dt, tag=f"{nm}{c}")
            nc.vector.tensor_copy(t[:], ps[:])
            dstlist.append(t)

    transpose_pair(sa_sb, saT, "saT")

    # b1d [128,1] = (e,k0) bias column via a K=1 outer product
    ps_b1 = psS.tile([128, 128], F32, tag="pst")
    nc.tensor.matmul(ps_b1[0:128, 0:1], b1rep[:], ones1[:], start=True, stop=True)
    b1d = const.tile([128, 1], F32, tag="b1d")
    nc.vector.tensor_copy(b1d[:], ps_b1[0:128, 0:1])

    transpose_pair(sb_sb, sbT, "sbT")

    ps_a2 = psS.tile([128, 128], F32, tag="pst")
    nc.tensor.matmul(ps_a2[:], Wa_dup[0][:], saT[0][:], start=True, stop=False)
    nc.tensor.matmul(ps_a2[:], Wa_dup[1][:], saT[1][:], start=False, stop=True)
    # a2rep [128, 512=(a,jj)] = (a_part + b1) repeated 4x along jj, via one
    # ACT eviction with a broadcast source
    a2rep = const.tile([128, 512], mm_dt, tag="a2rep")
    nc.scalar.activation(
        a2rep[:].rearrange("p (a j) -> p a j", j=4),
        ps_a2[:].unsqueeze(2).broadcast_to([128, 128, 4]),
        AF.Identity, bias=b1d[:, 0:1])

    ps_bT = psS.tile([64, 128], F32, tag="pst")
    nc.tensor.matmul(ps_bT[:], Wbh[:, 0:64], sbT[0][:], start=True, stop=False)
    nc.tensor.matmul(ps_bT[:], Wbh[:, 64:128], sbT[1][:], start=False, stop=True)
    # b_pairs2 [128=(e,k0), 64=(t,u,jj)]: col q' = 8t+4u+jj, j = 16t+4jj+2u+e
    b_pairs2 = const.tile([128, 64], mm_dt, tag="bpairs2")
    bTv = ps_bT[:].rearrange("p (t j u e) -> p t u j e", t=8, j=4, u=2)
    for e in (0, 1):
        nc.vector.tensor_copy(
            b_pairs2[64 * e : 64 * e + 64, :].rearrange(
                "p (t u j) -> p t u j", t=8, u=2),
            bTv[:, :, :, :, e])

    # b2q [128,1] = (u,e,k1) bias column
    ps_b2 = psS.tile([128, 128], F32, tag="pst")
    nc.tensor.matmul(ps_b2[0:128, 0:1], b2rep[:], ones1[:], start=True, stop=True)
    b2q = const.tile([128, 1], F32, tag="b2q")
    nc.vector.tensor_copy(b2q[:], ps_b2[0:128, 0:1])

    # ---------- psO seeds (b3 rows); s0/s1 in the head, s2/s3 in-loop ------
    psO_list = [None] * 4

    def seed(s):
        psO = psC.tile([128, 512], F32, tag="psO")
        nc.tensor.matmul(psO[:], ones_col[:], b3rep[:],
                         start=True, stop=False, skip_group_check=True)
        psO_list[s] = psO

    seed(0)
    seed(1)

    # ---------- L1 chunks: h1 = a2 (+) b (TT, 2x), n' = relu(-.99 h1) (TS, 4x)
    def emit_chunk(ci):
        h1 = h1pool.tile([128, 2048], mm_dt, tag="h1")
        in0 = (a2rep[:].rearrange("p (a j) -> p a j", j=4)
               .unsqueeze(1).broadcast_to([128, 4, 128, 4]))
        in1 = (b_pairs2[:, 16 * ci : 16 * ci + 16]
               .rearrange("p (g j) -> p g j", g=4)
               .unsqueeze(2).broadcast_to([128, 4, 128, 4]))
        nc.vector.tensor_tensor(
            out=h1[:].rearrange("p (g a j) -> p g a j", g=4, j=4),
            in0=in0, in1=in1, op=ALU.add)
        np_ = h1pool.tile([128, 2048], mm_dt, tag="nprime")
        nc.vector.tensor_scalar(out=np_[:], in0=h1[:], scalar1=neg_slope,
                                scalar2=0.0, op0=ALU.mult, op1=ALU.max)
        return h1, np_

    chunks = {0: emit_chunk(0), 1: emit_chunk(1)}

    out_flat = out.rearrange("a j k -> a (j k)")

    def l3_stage(t_, h2):
        psO = psO_list[t_ >> 1]
        h2v = h2[:].rearrange("p (a j) -> p j a", j=4)
        for jj in range(4):
            col = 256 * (t_ & 1) + 64 * jj
            last = (t_ % 2 == 1) and (jj == 3)
            nc.tensor.matmul(
                psO[:, col : col + 64], h2v[:, jj, :], W3dd[:],
                start=False, stop=last, skip_group_check=True)
        if t_ % 2 == 1:
            s = t_ >> 1
            osb = work.tile([128, 512], F32, tag="osb")
            if s < 3:
                act_leaky(osb[:], psO[:], 0.0)
                nc.sync.dma_start(out_flat[:, 512 * s : 512 * (s + 1)], osb[:])
            else:
                # split the tail eviction across ACT and DVE, and its DMA
                # across two queues
                act_leaky(osb[:, 0:256], psO[:, 0:256], 0.0)
                nc.vector.tensor_copy(osb[:, 256:512], psO[:, 256:512])
                nc.vector.scalar_tensor_tensor(
                    out=osb[:, 256:512], in0=osb[:, 256:512], scalar=alpha,
                    in1=osb[:, 256:512], op0=ALU.mult, op1=ALU.max)
                nc.sync.dma_start(out_flat[:, 1536:1792], osb[:, 0:256])
                nc.gpsimd.dma_start(out_flat[:, 1792:2048], osb[:, 256:512])

    # ---------- main L2 / L3 loop ----------
    h2_tiles = {}
    for t_ in range(8):
        ci, base = CHUNK_OF_T[t_]
        h1c, npc = chunks[ci]
        ps2 = psA.tile([128, 512], F32, tag="ps2")
        for u in (0, 1):
            nc.tensor.matmul(
                ps2[64 * u : 64 * u + 64, :], W2dd[:],
                h1c[:, base + 512 * u : base + 512 * u + 512],
                start=True, stop=False)
        if t_ == 0:
            seed(2)
        if t_ == 2:
            seed(3)
        for u in (0, 1):
            nc.tensor.matmul(
                ps2[64 * u : 64 * u + 64, :], W2dd[:],
                npc[:, base + 512 * u : base + 512 * u + 512],
                start=False, stop=True)
        if t_ == 1:
            chunks[2] = emit_chunk(2)
        if t_ == 3:
            chunks[3] = emit_chunk(3)
        h2 = work.tile([128, 512], mm_dt, tag="h2")
        act_leaky(h2[:], ps2[:], b2q[:, 0:1])
        h2_tiles[t_] = h2
        if t_ >= 2:
            l3_stage(t_ - 2, h2_tiles.pop(t_ - 2))
    l3_stage(6, h2_tiles.pop(6))
    l3_stage(7, h2_tiles.pop(7))


_NC_CACHE = {}


def _get_nc():
    if "nc" not in _NC_CACHE:
        _NC_CACHE["nc"] = build_nc()
    return _NC_CACHE["nc"]


def kernel(sent_a, sent_b, W1, b1, W2, b2, W3, b3):
    sent_a = np.ascontiguousarray(np.asarray(sent_a, dtype=np.float32))
    sent_b = np.ascontiguousarray(np.asarray(sent_b, dtype=np.float32))
    W1 = np.ascontiguousarray(np.asarray(W1, dtype=np.float32))
    b1 = np.ascontiguousarray(np.asarray(b1, dtype=np.float32))
    W2 = np.ascontiguousarray(np.asarray(W2, dtype=np.float32))
    b2 = np.ascontiguousarray(np.asarray(b2, dtype=np.float32))
    W3 = np.ascontiguousarray(np.asarray(W3, dtype=np.float32))
    b3 = np.ascontiguousarray(np.asarray(b3, dtype=np.float32))

    nc = _get_nc()
    in_maps = [{
        "sent_a": sent_a[i], "sent_b": sent_b[i],
        "W1": W1, "b1": b1, "W2": W2, "b2": b2, "W3": W3, "b3": b3,
    } for i in range(8)]
    res = bass_utils.run_bass_kernel_spmd(nc, in_maps, core_ids=list(range(8)))
    return np.stack([res.results[i]["out"] for i in range(8)]).astype(np.float32)
